# revision 10
# baseline (speedup 1.0000x reference)
"""DualModeSinkhorn Trainium2 kernel.

Problem (hardcoded from spec):
  log_H_res: (32, 64, 128, 128) f32, gaussian_kernel: (1,1,5,5) f32.
  H = exp(log_H_res) viewed as (B, 8, 8, H, W); 20 Sinkhorn iterations:
    row = smooth(sum_j H); H /= (row + eps)   [broadcast over j]
    col = smooth(sum_i H); H /= (col + eps)   [broadcast over i]
  smooth = circular 5x5 conv with the gaussian kernel, per (batch, channel).
  out = H + eps  (reference computes exp(log(H+eps)) == H+eps up to rounding)

Sharding: data-parallel over batch across 8 cores (4 batch elems per core).

Per-core layout: Hl state tile (p=h:128, free=(i,j,w):8x8x128) f32 resident in
SBUF for each of 4 batch elements.  Per half-iteration:
  - channel reduce on VectorE (tensor_reduce over strided axis)
  - 5x5 circular conv as 5 PSUM-accumulated matmuls with banded circulant
    lhsT matrices (one per kernel column); W-wrap via 132-wide halo tile,
    H-wrap encoded in the circulant.
  - 1/(x+eps) as exp(-ln(x+eps)) on ScalarE
  - broadcast multiply into Hl split between VectorE and GPSIMD
"""

import numpy as np

B, NCH, H, W = 32, 64, 128, 128
N = 8
KS = 5
PAD = 2
EPS = 1e-8
SINKHORN_ITER = 20
N_CORES = 8
PER_CORE = B // N_CORES  # 4

# i-slabs (of 8) whose broadcast-multiply goes to GPSIMD; rest on VectorE.
GPSIMD_SLABS = 5

_CACHE = {}


def _build_circulants(K):
    """lhsT_t[h, h'] = K[(h - h' + PAD) % H, t]  (zero outside the 5-band).

    Then smooth(S)[h', w'] = sum_t (lhsT_t.T @ S_shift_t)[h', w'] where
    S_shift_t is S with the w-axis circularly shifted by (t - PAD).
    """
    cht = np.zeros((KS, H, H), dtype=np.float32)
    for t in range(KS):
        for a in range(KS):
            for hp in range(H):
                cht[t, (hp + a - PAD) % H, hp] = K[a, t]
    return cht


def _split_waits(nc):
    """This walrus build allows only ONE sync-wait per instruction; Tile
    emits several.  Hoist extras onto same-engine EventSemaphore nops
    inserted immediately before the offending instruction."""
    import bass_rust as br
    from concourse import mybir

    for f in nc.m.functions:
        for bb in f.blocks:
            insts = list(bb.instructions)
            for idx in range(len(insts) - 1, -1, -1):
                ins = insts[idx]
                si = ins.sync_info
                if si is None or len(si.on_wait) <= 1:
                    continue
                waits = list(si.on_wait)
                ins.sync_info = br.SyncInfo(
                    on_wait=[waits[-1]], on_update=list(si.on_update)
                )
                for w in waits[:-1]:
                    nop = mybir.InstEventSemaphore(
                        name=f"waitsplit-{nc.next_id()}", ins=[], outs=[]
                    )
                    nop.engine = ins.engine
                    nop.sync_info = br.SyncInfo(on_wait=[w], on_update=[])
                    nc.register_instruction(nop, overwrite=True)
                    bb.instructions.insert(idx, nop)


def _build_nc(n_iters, n_batch):
    import concourse.bass as bass
    import concourse.tile as tile
    from concourse import mybir

    f32 = mybir.dt.float32
    nc = bass.Bass("TRN2", target_bir_lowering=False, debug=False)

    x_d = nc.dram_tensor("x", [n_batch, NCH, H, W], f32, kind="ExternalInput")
    cht_d = nc.dram_tensor("cht", [KS, H, H], f32, kind="ExternalInput")
    y_d = nc.dram_tensor("y", [n_batch, NCH, H, W], f32, kind="ExternalOutput")

    x_ap = x_d.ap()
    cht_ap = cht_d.ap()
    y_ap = y_d.ap()

    P = 128
    WH = W + 2 * PAD  # halo width 132

    with tile.TileContext(nc) as tc:
        with (
            tc.tile_pool(name="state", bufs=1) as state,
            tc.tile_pool(name="consts", bufs=1) as consts,
            tc.tile_pool(name="stage", bufs=8) as stagep,
            tc.tile_pool(name="marg", bufs=4) as margp,
            tc.tile_pool(name="recip", bufs=4) as recipp,
            tc.tile_pool(name="psum", bufs=3, space="PSUM") as psump,
        ):
            # constants
            cht_t = []
            for t in range(KS):
                ct = consts.tile([P, H], f32, tag=f"cht{t}")
                nc.sync.dma_start(out=ct[:], in_=cht_ap[t])
                cht_t.append(ct)
            eps_t = consts.tile([P, 1], f32, tag="eps")
            nc.vector.memset(eps_t[:], EPS)

            # per-batch state tiles + load + exp
            hls = []
            for b in range(n_batch):
                hl = state.tile([P, N, N, W], f32, tag=f"hl{b}")
                hls.append(hl)
                src = x_ap[b].transpose([1, 0, 2])  # (h, c, w)
                for i in range(N):
                    st = stagep.tile([P, N, W], f32, tag="stage")
                    nc.sync.dma_start(
                        out=st[:],
                        in_=src[:, i * N:(i + 1) * N, :],
                    )
                    nc.scalar.activation(
                        out=hl[:, i, :, :].rearrange("p j w -> p (j w)"),
                        in_=st[:].rearrange("p j w -> p (j w)"),
                        func=mybir.ActivationFunctionType.Exp,
                    )

            def half_iter(hl, kind):
                # ---- marginal reduce into halo tile ----
                marg = margp.tile([P, N, WH], f32, tag="marg")
                hv = hl[:]  # (p, i, j, w)
                if kind == "row":
                    red_in = hv.transpose([0, 1, 3, 2])  # (p, i, w, j)
                else:
                    red_in = hv.transpose([0, 2, 3, 1])  # (p, j, w, i)
                nc.vector.tensor_reduce(
                    out=marg[:, :, PAD:PAD + W],
                    in_=red_in,
                    axis=mybir.AxisListType.X,
                    op=mybir.AluOpType.add,
                )
                # circular halo columns
                nc.vector.tensor_copy(
                    out=marg[:, :, 0:PAD], in_=marg[:, :, W:W + PAD]
                )
                nc.vector.tensor_copy(
                    out=marg[:, :, W + PAD:W + 2 * PAD],
                    in_=marg[:, :, PAD:2 * PAD],
                )

                # ---- 5x5 circular conv: 5 accumulated matmuls per half ----
                ps = psump.tile([P, N, W], f32, tag="ps")
                for half in range(2):
                    c0 = half * (N // 2)
                    for t in range(KS):
                        nc.tensor.matmul(
                            out=ps[:, c0:c0 + N // 2, :],
                            lhsT=cht_t[t][:],
                            rhs=marg[:, c0:c0 + N // 2, t:t + W],
                            start=(t == 0),
                            stop=(t == KS - 1),
                        )

                # ---- reciprocal of (conv + eps): exp(-ln(x + eps)) ----
                rec = recipp.tile([P, N, W], f32, tag="rec")
                nc.scalar.activation(
                    out=rec[:].rearrange("p c w -> p (c w)"),
                    in_=ps[:].rearrange("p c w -> p (c w)"),
                    func=mybir.ActivationFunctionType.Ln,
                    bias=eps_t[:],
                    scale=1.0,
                )
                nc.scalar.activation(
                    out=rec[:].rearrange("p c w -> p (c w)"),
                    in_=rec[:].rearrange("p c w -> p (c w)"),
                    func=mybir.ActivationFunctionType.Exp,
                    scale=-1.0,
                )

                # ---- broadcast multiply into Hl (split DVE / GPSIMD) ----
                if kind == "row":
                    rec_b = rec[:].unsqueeze(2).broadcast_to([P, N, N, W])
                else:
                    rec_b = rec[:].unsqueeze(1).broadcast_to([P, N, N, W])
                k = GPSIMD_SLABS
                nc.vector.tensor_mul(
                    out=hv[:, : N - k], in0=hv[:, : N - k], in1=rec_b[:, : N - k]
                )
                if k:
                    nc.gpsimd.tensor_mul(
                        out=hv[:, N - k:], in0=hv[:, N - k:], in1=rec_b[:, N - k:]
                    )

            for it in range(n_iters):
                for b in range(n_batch):
                    half_iter(hls[b], "row")
                    half_iter(hls[b], "col")

            # ---- epilogue: +eps, DMA out ----
            for b in range(n_batch):
                hl = hls[b]
                dst = y_ap[b].transpose([1, 0, 2])  # (h, c, w)
                for i in range(N):
                    ot = stagep.tile([P, N, W], f32, tag="stage")
                    nc.scalar.activation(
                        out=ot[:].rearrange("p j w -> p (j w)"),
                        in_=hl[:, i, :, :].rearrange("p j w -> p (j w)"),
                        func=mybir.ActivationFunctionType.Identity,
                        bias=eps_t[:],
                        scale=1.0,
                    )
                    nc.sync.dma_start(
                        out=dst[:, i * N:(i + 1) * N, :],
                        in_=ot[:],
                    )
    _split_waits(nc)
    return nc


def _get_nc(n_iters, n_batch):
    key = (n_iters, n_batch)
    if key not in _CACHE:
        _CACHE[key] = _build_nc(n_iters, n_batch)
    return _CACHE[key]


def _default_gaussian_kernel():
    # mirror reference._gaussian_kernel (KERNEL_SIZE=5, BANDWIDTH=0.5) in fp32
    x = (np.arange(KS, dtype=np.float32) - KS // 2)
    xx, yy = np.meshgrid(x, x, indexing="ij")
    k = np.exp(-(xx**2 + yy**2) / np.float32(2.0 * 0.5**2)).astype(np.float32)
    return (k / k.sum()).astype(np.float32)


def kernel(log_H_res, gaussian_kernel=None, _n_iters=SINKHORN_ITER, _trace=False):
    from concourse.bass_utils import run_bass_kernel_spmd

    x = np.ascontiguousarray(np.asarray(log_H_res, dtype=np.float32))
    if gaussian_kernel is None:
        K = _default_gaussian_kernel()
    else:
        K = np.asarray(gaussian_kernel, dtype=np.float32).reshape(KS, KS)
    cht = _build_circulants(K)

    nc = _get_nc(_n_iters, PER_CORE)

    in_maps = []
    for c in range(N_CORES):
        in_maps.append(
            {
                "x": x[c * PER_CORE:(c + 1) * PER_CORE],
                "cht": cht,
            }
        )
    res = run_bass_kernel_spmd(
        nc, in_maps, core_ids=list(range(N_CORES)), trace=_trace
    )
    out = np.concatenate([r["y"] for r in res.results], axis=0)
    out = out.reshape(B, N, N, H, W).astype(np.float32)
    if _trace:
        kernel._last_results = res
    return out
